# revision 1
# baseline (speedup 1.0000x reference)
"""Causal self-attention (d_model=1024, n_head=16, seq=4096) on 8 trn2 cores.

Sharding: tensor-parallel over heads (2 heads/core) for QKV + attention,
then an AllToAll re-shards y^T from head-sharded to sequence-sharded, so
each core runs the output projection for seq/8 rows with the full w_proj
(no AllReduce). The host concatenates the 8 row-shards.

Per-core layout (bf16 into the PE, fp32 PSUM accumulation):
  - x^T built via PE identity-matmul transposes (the d_model contraction
    needs x in [c, T] layout for both qkv operands).
  - qkv^T = w_slice.T @ x^T lands directly in [chan, T] layout, so qT/kT
    are exactly the lhsT/rhs of the score matmul (scores^T = K Q^T), and
    V' (normal orientation + a ones column) comes from small PE transposes.
  - softmax without max-subtraction (scores ~ N(0,1): exp cannot overflow
    fp32); the denominator falls out of the y^T matmul as the ones-column
    row; normalization uses exp(-ln(denom)) on ScalarE plus a K=1 matmul
    to broadcast the reciprocal across partitions.
  - causal masking: only lower-triangle k-tiles are computed; diagonal
    tiles are masked by a precomputed 0/1 multiply after the exp.
  - emission is braided: prep for block n+1 (x load/transpose/qkv/V') is
    interleaved between the attention groups of q-block n, under a single
    shared PSUM pool, so PE/ACT/DVE/DMA overlap across phases.
"""

import sys
import types

import numpy as np
import ml_dtypes

D_MODEL = 1024
N_HEAD = 16
SEQ = 4096
N_CORES = 8
D_HEAD = 64
CPC = 128            # channels per core (2 heads x 64)
QB = 512             # attention q-block width
BF16 = ml_dtypes.bfloat16


def _install_compat_patches():
    """Stub antenv.axon_hooks (absent in this container) so
    run_bass_kernel_spmd's trace path degrades instead of ImportError."""
    if "antenv.axon_hooks" not in sys.modules:
        mod = types.ModuleType("antenv.axon_hooks")
        mod.get_axon_ntff_profile_hook = lambda: None
        sys.modules["antenv.axon_hooks"] = mod


def _split_multi_waits(nc):
    """The nix walrus here accepts at most ONE sync-wait per instruction
    (setupSyncWait: 'Too many sync wait commands').  Hoist extra waits onto
    same-engine NoOps inserted immediately before the instruction — engine
    streams execute in program order, so semantics are unchanged."""
    import concourse.mybir as mybir

    n = 0
    for fn in nc.m.functions:
        for bb in fn.blocks:
            insts = bb.instructions
            out = []
            for inst in insts:
                si = getattr(inst, "sync_info", None)
                waits = list(si.on_wait) if si is not None else []
                if len(waits) > 1:
                    si.on_wait.clear()
                    for w in waits[:-1]:
                        n += 1
                        nop = mybir.InstNoOp(name=f"I-WSPLIT{n}", ins=[], outs=[])
                        nop.engine = inst.engine
                        nop.sync_info = mybir.SyncInfo(on_wait=[w], on_update=[])
                        out.append(nop)
                    si.on_wait.append(waits[-1])
                out.append(inst)
            bb.instructions = out


def build_nc(seq=SEQ, use_collective=True, split_waits=True):
    """Build the single-core SPMD program (identical on all 8 cores)."""
    import concourse.bass as bass
    import concourse.mybir as mybir
    from concourse.tile import TileContext

    _install_compat_patches()

    f32 = mybir.dt.float32
    bf16 = mybir.dt.bfloat16
    AFT = mybir.ActivationFunctionType

    nT = seq // 128       # T-tiles
    nQB = seq // QB       # attention q-blocks
    SW = seq // N_CORES   # AllToAll shard width (output rows per core)

    nc = bass.Bass("TRN2", target_bir_lowering=False, debug=False,
                   num_devices=N_CORES)
    x_d = nc.dram_tensor("x", [seq, D_MODEL], f32, kind="ExternalInput").ap()
    wq_d = nc.dram_tensor("w_slice", [D_MODEL, 3 * CPC], f32,
                          kind="ExternalInput").ap()
    wp_d = nc.dram_tensor("w_proj", [D_MODEL, D_MODEL], f32,
                          kind="ExternalInput").ap()
    id_d = nc.dram_tensor("ident", [128, 128], bf16, kind="ExternalInput").ap()
    mk_d = nc.dram_tensor("masks", [4, 128, QB], bf16,
                          kind="ExternalInput").ap()
    out_d = nc.dram_tensor("out", [SW, D_MODEL], f32,
                           kind="ExternalOutput").ap()

    with TileContext(nc) as tc:
        with (
            tc.tile_pool(name="per", bufs=1) as per,
            tc.tile_pool(name="stg", bufs=2) as stg,
            tc.tile_pool(name="dram", bufs=1, space="DRAM") as dram,
        ):
            qT = per.tile([128, seq], bf16)      # [2 heads x 64 d, T]
            kT = per.tile([128, seq], bf16)
            Vp = per.tile([128, nT, 130], bf16)  # V' tiles: [v_h0|1|v_h1|1]
            yn0 = per.tile([64, seq], bf16)      # normalized y^T, head 0
            yn1 = per.tile([64, seq], bf16)
            wqkv = per.tile([128, 8, 3 * CPC], bf16)
            wpj = per.tile([128, 8, D_MODEL], bf16)
            iden = per.tile([128, 128], bf16)
            mks = per.tile([128, 4, QB], bf16)
            ones = per.tile([128, 64], f32)
            a2a_sb = per.tile([128, 8, SW], bf16)

            nc.sync.dma_start(iden[:], id_d[:])
            for m in range(4):
                nc.sync.dma_start(mks[:, m, :], mk_d[m])
            nc.any.memset(ones[:], 1.0)
            nc.any.memset(Vp[:, :, 64:65], 1.0)
            nc.any.memset(Vp[:, :, 129:130], 1.0)

            # (weight staging happens inside the xstg pool below)

            a2a_in = dram.tile([N_CORES * CPC, SW], bf16)
            a2a_out = dram.tile([N_CORES * CPC, SW], bf16)

            # ---- phases 0-2, braided emission ------------------------
            # Engines execute their scheduled streams in static order, so
            # overlap must be built into emission order: the prep work
            # (x-load/transpose/qkv/V') for block n+1 is interleaved chunk-
            # by-chunk between the attention groups of q-block n.  Attention
            # qb=n depends only on qkv blocks 0..n, so each braid is legal.
            # PSUM banks: pA 2x1 + sT 2x2 + yt0 1 + yt1 1 = 8
            with (
                tc.tile_pool(name="xp", bufs=1) as xp,
                tc.tile_pool(name="xstg", bufs=3) as xstg,
                tc.tile_pool(name="ps", bufs=2, space="PSUM") as ps,
            ):
                xT = xp.tile([128, 8, seq], bf16)   # [c-chunk part, chunk, T]

                def wqkv_stage():
                    for k in range(8):
                        wtmp = xstg.tile([128, 3 * CPC], f32, tag="xf",
                                         bufs=3, name=f"wtmp_{k}")
                        nc.sync.dma_start(wtmp[:],
                                          wq_d[128 * k:128 * (k + 1), :])
                        nc.vector.tensor_copy(wqkv[:, k, :], wtmp[:])

                def prep_chunks(n):
                    """Emit-closures for block n: loads, x^T, qkv^T, V'."""
                    state = {}

                    def loads():
                        xbs = []
                        for u in range(4):
                            t = 4 * n + u
                            xf = xstg.tile([128, D_MODEL], f32, tag="xf",
                                           bufs=3, name=f"xf_{t}")
                            nc.sync.dma_start(xf[:],
                                              x_d[128 * t:128 * (t + 1), :])
                            xb = xstg.tile([128, D_MODEL], bf16, tag="xb",
                                           bufs=4, name=f"xb_{t}")
                            nc.vector.tensor_copy(xb[:], xf[:])
                            xbs.append(xb)
                        state["xbs"] = xbs

                    def trans(j):
                        # j indexes (x-tile u = j//2, c-chunk quad a = j%2):
                        # one PSUM tile holds 4 c-chunk transposes of a
                        # single x-tile, so work starts after its one load
                        def emit():
                            u, a = divmod(j, 2)
                            tp = ps.tile([128, 512], f32, tag="pA",
                                         name=f"tp_{n}_{j}")
                            for c in range(4):
                                nc.tensor.matmul(
                                    tp[:, 128 * c:128 * (c + 1)],
                                    state["xbs"][u][:, 128 * (4 * a + c):
                                                    128 * (4 * a + c + 1)],
                                    iden[:], start=True, stop=True)
                            nc.vector.tensor_copy(
                                xT[:, 4 * a:4 * (a + 1),
                                   128 * (4 * n + u):128 * (4 * n + u + 1)],
                                tp[:])
                        return emit

                    def qkv(m):
                        def emit():
                            qp = ps.tile([128, 512], f32, tag="pA",
                                         name=f"qp_{n}_{m}")
                            for k in range(8):
                                nc.tensor.matmul(
                                    qp[:],
                                    wqkv[:, k, 128 * m:128 * (m + 1)],
                                    xT[:, k, 512 * n:512 * (n + 1)],
                                    start=(k == 0), stop=(k == 7))
                            if m == 0:
                                nc.vector.tensor_copy(
                                    qT[:, 512 * n:512 * (n + 1)], qp[:])
                            elif m == 1:
                                nc.vector.tensor_copy(
                                    kT[:, 512 * n:512 * (n + 1)], qp[:])
                            else:
                                vs = xstg.tile([128, 512], bf16, tag="vs",
                                               bufs=2, name=f"vs_{n}")
                                nc.vector.tensor_copy(vs[:], qp[:])
                                state["vs"] = vs
                        return emit

                    def vtr(u):
                        def emit():
                            t = 4 * n + u
                            vs = state["vs"]
                            # separate PSUM tiles per head: PE-write plus
                            # DVE-read of one PSUM bank is a HW fault
                            vp0 = ps.tile([128, 64], f32, tag="pA",
                                          name=f"vp0_{t}")
                            vp1 = ps.tile([128, 64], f32, tag="pA",
                                          name=f"vp1_{t}")
                            nc.tensor.matmul(
                                vp0[:], vs[0:64, 128 * u:128 * (u + 1)],
                                iden[0:64, 0:64], start=True, stop=True)
                            nc.tensor.matmul(
                                vp1[:], vs[64:128, 128 * u:128 * (u + 1)],
                                iden[64:128, 64:128], start=True, stop=True)
                            nc.vector.tensor_copy(Vp[:, t, 0:64], vp0[:])
                            nc.vector.tensor_copy(Vp[:, t, 65:129], vp1[:])
                        return emit

                    return ([loads] + [trans(j) for j in range(8)]
                            + [qkv(m) for m in range(3)]
                            + [vtr(u) for u in range(4)])

                def attention_groups(qb, ytps):
                    nkt = 4 * (qb + 1)

                    def group(g):
                        # diagonal k-tiles (d = kt-4qb >= 0) only attend to
                        # q >= 128d: trim score MM / exp / mask / yT MM to
                        # the valid column range [128d, QB).  q-cols below
                        # that are fully masked and, because kt=0 always
                        # covers the full width with start=True, never read.
                        def off(kt):
                            d = kt - 4 * qb
                            return 128 * d if d >= 0 else 0

                        def emit():
                            # h-inner MM order: consecutive score matmuls use
                            # disjoint PE row-groups (h0 rows 0-63, h1 rows
                            # 64-127) so the 16x32x32-subarray PE overlaps
                            # them (K=64 packing, ~2x on the score matmuls)
                            sps = [ps.tile([128, 2 * QB], f32, tag="sT",
                                           name=f"sp_{qb}_{g}_{h}")
                                   for h in (0, 1)]
                            for u in (0, 1):
                                kt = 2 * g + u
                                o = off(kt)
                                for h in (0, 1):
                                    nc.tensor.matmul(
                                        sps[h][:, QB * u + o:QB * (u + 1)],
                                        kT[64 * h:64 * (h + 1),
                                           128 * kt:128 * (kt + 1)],
                                        qT[64 * h:64 * (h + 1),
                                           QB * qb + o:QB * (qb + 1)],
                                        start=True, stop=True)
                            diag = off(2 * g) > 0 or off(2 * g + 1) > 0
                            for h in (0, 1):
                                pt = stg.tile([128, 2 * QB], bf16, tag="pT",
                                              bufs=3, name=f"pt_{qb}_{g}_{h}")
                                if diag:
                                    for u in (0, 1):
                                        o = off(2 * g + u)
                                        nc.scalar.activation(
                                            pt[:, QB * u + o:QB * (u + 1)],
                                            sps[h][:, QB * u + o:QB * (u + 1)],
                                            AFT.Exp, scale=0.125)
                                else:
                                    nc.scalar.activation(pt[:], sps[h][:],
                                                         AFT.Exp, scale=0.125)
                                for u in (0, 1):
                                    kt = 2 * g + u
                                    d = kt - 4 * qb
                                    o = off(kt)
                                    if d >= 0:
                                        nc.vector.tensor_mul(
                                            pt[:, QB * u + o:QB * (u + 1)],
                                            pt[:, QB * u + o:QB * (u + 1)],
                                            mks[:, d, o:QB])
                                    nc.tensor.matmul(
                                        ytps[h][:, o:QB],
                                        Vp[:, kt, 65 * h:65 * (h + 1)],
                                        pt[:, QB * u + o:QB * (u + 1)],
                                        start=(kt == 0),
                                        stop=(kt == nkt - 1))
                        return emit

                    return [group(g) for g in range(nkt // 2)]

                def normalize(qb, ytps):
                    for h in (0, 1):
                        # one copy frees the PSUM accumulator right away; the
                        # denom -> 1/denom -> broadcast -> scale chain then
                        # runs from SBUF off the critical path.
                        yu = stg.tile([65, 2 * QB], f32, tag="dn", bufs=4,
                                      name=f"yu_{qb}_{h}")
                        nc.vector.tensor_copy(yu[:, 0:QB], ytps[h][:])
                        nc.scalar.activation(yu[64:65, QB:2 * QB],
                                             yu[64:65, 0:QB], AFT.Ln)
                        nc.scalar.activation(yu[64:65, QB:2 * QB],
                                             yu[64:65, QB:2 * QB], AFT.Exp,
                                             scale=-1.0)
                        bcp = ps.tile([64, QB], f32, tag="pA",
                                      name=f"bcp_{qb}_{h}")
                        nc.tensor.matmul(bcp[:], ones[64:65, 0:64],
                                         yu[64:65, QB:2 * QB],
                                         start=True, stop=True)
                        bcs = stg.tile([64, QB], f32, tag="bcs", bufs=2,
                                       name=f"bcs_{qb}_{h}")
                        nc.vector.tensor_copy(bcs[:], bcp[:])
                        yn = yn0 if h == 0 else yn1
                        nc.vector.tensor_mul(yn[:, QB * qb:QB * (qb + 1)],
                                             yu[0:64, 0:QB], bcs[:])
                        if SW == QB:
                            # q-block == shard: stage its AllToAll rows now
                            j = qb
                            r0 = 128 * j + 64 * h
                            nc.sync.dma_start(a2a_in[r0:r0 + 64, :],
                                              yn[:, SW * j:SW * (j + 1)])

                def wpj_chunk(k):
                    def emit():
                        # w_proj staged late (projection tail only) and
                        # braided into the final attention block, which has
                        # no other prep work to overlap with
                        ptmp = xstg.tile([128, D_MODEL], f32, tag="xf",
                                         bufs=3, name=f"ptmp_{k}")
                        nc.sync.dma_start(ptmp[:],
                                          wp_d[128 * k:128 * (k + 1), :])
                        nc.vector.tensor_copy(wpj[:, k, :], ptmp[:])
                    return emit

                p0 = prep_chunks(0)
                p0[0]()           # stage-0 x loads lead the DMA queues
                wqkv_stage()
                for c in p0[1:]:
                    c()
                for n in range(nQB):
                    ytps = [ps.tile([65, QB], f32, tag=f"yt{h}", bufs=1,
                                    name=f"yt{h}_{n}") for h in (0, 1)]
                    if n + 1 < nQB:
                        pend = prep_chunks(n + 1)
                    else:
                        pend = [wpj_chunk(k) for k in range(8)]
                    groups = attention_groups(n, ytps)
                    ci = 0
                    for gi, g in enumerate(groups):
                        g()
                        want = (gi + 1) * len(pend) // len(groups)
                        while ci < want:
                            pend[ci]()
                            ci += 1
                    while ci < len(pend):
                        pend[ci]()
                        ci += 1
                    normalize(n, ytps)

            # ---- phase 3: AllToAll head-shard -> seq-shard ----------------
            if SW != QB:
                for j in range(N_CORES):
                    nc.sync.dma_start(a2a_in[128 * j:128 * j + 64, :],
                                      yn0[:, SW * j:SW * (j + 1)])
                    nc.sync.dma_start(a2a_in[128 * j + 64:128 * (j + 1), :],
                                      yn1[:, SW * j:SW * (j + 1)])
            if use_collective:
                nc.gpsimd.collective_compute(
                    "AllToAll", mybir.AluOpType.bypass,
                    ins=[a2a_in.opt()], outs=[a2a_out.opt()],
                    replica_groups=[list(range(N_CORES))])
            else:
                # timing-model variant (TimelineSim can't simulate
                # collectives): stand-in DRAM->DRAM copy
                nc.sync.dma_start(a2a_out[:], a2a_in[:])
            for j in range(N_CORES):
                nc.sync.dma_start(a2a_sb[:, j, :],
                                  a2a_out[128 * j:128 * (j + 1), :])

            # ---- phase 4: output projection for this core's SW rows -------
            with tc.tile_pool(name="psC", bufs=2, space="PSUM") as psC:
                mw = min(128, SW)
                for m in range(SW // mw):
                    pp = psC.tile([mw, D_MODEL], f32, tag="pp")
                    for n2 in (0, 1):
                        for k in range(8):
                            nc.tensor.matmul(
                                pp[:, 512 * n2:512 * (n2 + 1)],
                                a2a_sb[:, k, mw * m:mw * (m + 1)],
                                wpj[:, k, 512 * n2:512 * (n2 + 1)],
                                start=(k == 0), stop=(k == 7))
                    ob = stg.tile([mw, D_MODEL], f32, tag="ob", bufs=2)
                    nc.vector.tensor_copy(ob[:], pp[:])
                    nc.sync.dma_start(out_d[mw * m:mw * (m + 1), :], ob[:])

    if split_waits:
        _split_multi_waits(nc)
    return nc


def make_aux_inputs():
    ident = np.eye(128, dtype=BF16)
    k_idx = np.arange(128)[:, None]
    q_idx = np.arange(QB)[None, :]
    masks = np.stack(
        [((k_idx + 128 * d) <= q_idx).astype(BF16) for d in range(4)], axis=0)
    return ident, masks


def make_in_maps(x, w_qkv, w_proj, seq=SEQ):
    x = np.asarray(x, dtype=np.float32).reshape(seq, D_MODEL)
    w_qkv = np.asarray(w_qkv, dtype=np.float32)
    w_proj = np.asarray(w_proj, dtype=np.float32)
    ident, masks = make_aux_inputs()
    in_maps = []
    for i in range(N_CORES):
        sl = slice(CPC * i, CPC * (i + 1))
        w_slice = np.concatenate(
            [w_qkv[:, sl], w_qkv[:, D_MODEL:][:, sl],
             w_qkv[:, 2 * D_MODEL:][:, sl]], axis=1)
        in_maps.append({
            "x": x,
            "w_slice": np.ascontiguousarray(w_slice),
            "w_proj": w_proj,
            "ident": ident,
            "masks": masks,
        })
    return in_maps


_NC_CACHE = {}


def kernel(x, w_qkv, w_proj):
    """Full inputs in, full output out. Shards internally across 8 cores."""
    try:
        import os
        import jax
        jax.config.update("jax_compilation_cache_dir",
                          os.path.expanduser("~/.cache/jax_bass_kernel"))
        jax.config.update("jax_persistent_cache_min_compile_time_secs", 0.0)
    except Exception:
        pass
    from concourse.bass_utils import run_bass_kernel_spmd

    x = np.asarray(x, dtype=np.float32)
    batch = x.shape[0]
    seq = x.shape[1]
    if seq not in _NC_CACHE:
        _NC_CACHE[seq] = build_nc(seq)
    nc = _NC_CACHE[seq]
    in_maps = make_in_maps(x, w_qkv, w_proj, seq=seq)
    res = run_bass_kernel_spmd(nc, in_maps, list(range(N_CORES)))
    out = np.concatenate([res.results[j]["out"] for j in range(N_CORES)],
                         axis=0)
    return out.reshape(batch, seq, D_MODEL).astype(np.float32)



# revision 24
# speedup vs baseline: 1.4509x; 1.4509x over previous
"""Causal self-attention (d_model=1024, n_head=16, seq=4096) on 8 trn2 cores.

Sharding: tensor-parallel over heads (2 heads/core) for QKV + attention.
The output shard is q-tile interleaved: core j owns q-tiles {j, 8+j, 16+j,
24+j} (128 rows each), so the head->seq reshard runs as FOUR small
AllToAlls (one per q-tile group), each issued as soon as its two q-blocks
finish -- collectives #0-2 overlap the remaining attention, only #3 is
tail.  Each core projects its 4 q-tiles with the full w_proj; the host
re-interleaves the rows.

Per-core layout (bf16 into the PE, fp32 PSUM accumulation):
  - x^T, w_qkv slice, w_proj are pre-transposed/pre-cast to bf16 on the
    host (pure data movement), so no on-device transposes or dtype
    converts of inputs remain.
  - qkv^T = w_slice.T @ x^T lands in [chan, T] layout: qT/kT are directly
    the lhsT/rhs of the score matmul (scores^T = K Q^T); V' (natural
    orientation, with leading/trailing ones columns for the softmax
    denominators) comes from small PE transposes.
  - softmax without max-subtraction (scores ~ N(0,1): exp cannot overflow
    fp32); AV runs in natural orientation y[q,d] via lhsT=exp-scores
    (M=128 q rows per matmul, N=65), so the denominator falls out as a
    ones-column and normalization is one per-partition-scalar multiply
    after a DVE reciprocal.
  - causal masking: only lower-triangle k-tiles are computed; diagonal
    tiles are masked by a precomputed 0/1 multiply after the exp.
  - emission is braided: prep for block n+1 (x^T DMA/qkv/V') and the
    projection of already-landed AllToAll groups are interleaved between
    the attention groups of q-block n under a shared PSUM pool.
"""

import sys
import types

import numpy as np
import ml_dtypes

D_MODEL = 1024
N_HEAD = 16
SEQ = 4096
N_CORES = 8
D_HEAD = 64
CPC = 128            # channels per core (2 heads x 64)
QB = 512             # attention q-block width
NG = 4               # AllToAll groups (qt-interleaved output shard)
BF16 = ml_dtypes.bfloat16


def _install_compat_patches():
    """Stub antenv.axon_hooks (absent in this container) so
    run_bass_kernel_spmd's trace path degrades instead of ImportError."""
    if "antenv.axon_hooks" not in sys.modules:
        mod = types.ModuleType("antenv.axon_hooks")
        mod.get_axon_ntff_profile_hook = lambda: None
        sys.modules["antenv.axon_hooks"] = mod


def _split_multi_waits(nc):
    """The nix walrus here accepts at most ONE sync-wait per instruction
    (setupSyncWait: 'Too many sync wait commands').  Hoist extra waits onto
    same-engine NoOps inserted immediately before the instruction -- engine
    streams execute in program order, so semantics are unchanged."""
    import concourse.mybir as mybir

    n = 0
    for fn in nc.m.functions:
        for bb in fn.blocks:
            insts = bb.instructions
            out = []
            for inst in insts:
                si = getattr(inst, "sync_info", None)
                waits = list(si.on_wait) if si is not None else []
                if len(waits) > 1:
                    si.on_wait.clear()
                    for w in waits[:-1]:
                        n += 1
                        nop = mybir.InstNoOp(name=f"I-WSPLIT{n}", ins=[], outs=[])
                        nop.engine = inst.engine
                        nop.sync_info = mybir.SyncInfo(on_wait=[w], on_update=[])
                        out.append(nop)
                    si.on_wait.append(waits[-1])
                out.append(inst)
            bb.instructions = out


def build_nc(seq=SEQ, use_collective=True, split_waits=True, debug=None):
    """Build the single-core SPMD program (identical on all 8 cores)."""
    import concourse.bass as bass
    import concourse.mybir as mybir
    from concourse.tile import TileContext

    _install_compat_patches()

    f32 = mybir.dt.float32
    bf16 = mybir.dt.bfloat16
    AFT = mybir.ActivationFunctionType

    nT = seq // 128       # k-tiles
    nQB = seq // QB       # attention q-blocks (8)
    nQT = seq // 128      # q-tiles (32)
    assert nQB == 2 * NG

    nc = bass.Bass("TRN2", target_bir_lowering=False, debug=False,
                   num_devices=N_CORES)
    # host-prepped layouts: [part(128), chunk, free]
    x_d = nc.dram_tensor("xT", [128, 8, seq], bf16, kind="ExternalInput").ap()
    wq_d = nc.dram_tensor("w_slice", [128, 8, 3 * CPC], bf16,
                          kind="ExternalInput").ap()
    wp_d = nc.dram_tensor("w_proj", [128, 8, D_MODEL], bf16,
                          kind="ExternalInput").ap()
    id_d = nc.dram_tensor("ident", [128, 128], bf16, kind="ExternalInput").ap()
    mk_d = nc.dram_tensor("masks", [4, 128, QB], bf16,
                          kind="ExternalInput").ap()
    # out rows: group-major, 128 rows per group = this core's q-tile 8g+j
    out_d = nc.dram_tensor("out", [NG * 128, D_MODEL], f32,
                           kind="ExternalOutput").ap()

    with TileContext(nc) as tc:
        with (
            tc.tile_pool(name="per", bufs=1) as per,
            tc.tile_pool(name="stg", bufs=2) as stg,
            tc.tile_pool(name="dram", bufs=1, space="DRAM") as dram,
        ):
            xT = per.tile([128, 8, seq], bf16)
            qT = per.tile([128, seq], bf16)      # [2 heads x 64 d, T]
            kT = per.tile([128, seq], bf16)
            # V' tiles: [1 | v_h0 | v_h1 | 1]  (denoms ride the AV matmul)
            Vp = per.tile([128, nT, 130], bf16)
            wqkv = per.tile([128, 8, 3 * CPC], bf16)
            wpj = per.tile([128, 8, D_MODEL], bf16)
            iden = per.tile([128, 128], bf16)
            mks = per.tile([128, 4, QB], bf16)

            nc.any.memset(Vp[:, :, 0:1], 1.0)
            nc.any.memset(Vp[:, :, 129:130], 1.0)

            a2a_in = [dram.tile([N_CORES, 128, 128], bf16, name=f"a2a_in{g}")
                      for g in range(NG)]
            a2a_out = [dram.tile([N_CORES, 128, 128], bf16, name=f"a2a_out{g}")
                       for g in range(NG)]

            with (
                tc.tile_pool(name="ps", bufs=2, space="PSUM") as ps,
            ):
                # ---- braided emission ---------------------------------
                # PSUM banks: sT 2x2 + yt0 1 + yt1 1 + pA 2x1 = 8
                def xload(n):
                    def emit():
                        nc.sync.dma_start(xT[:, :, QB * n:QB * (n + 1)],
                                          x_d[:, :, QB * n:QB * (n + 1)])
                    return emit

                def wpj_load(half):
                    def emit():
                        nc.sync.dma_start(
                            wpj[:, 4 * half:4 * (half + 1), :],
                            wp_d[:, 4 * half:4 * (half + 1), :])
                    return emit

                def qkv(n, m):
                    """Split into fine closures: 2 k-chunk matmuls each, then
                    the PSUM->SBUF copy, so braiding never starves ACT."""
                    state = {}

                    def mm(k0):
                        def emit():
                            if debug == "prepnq":
                                return
                            if k0 == 0:
                                state["qp"] = ps.tile([128, QB], f32,
                                                      tag="pA",
                                                      name=f"qp_{n}_{m}")
                            qp = state["qp"]
                            for k in (k0, k0 + 1):
                                nc.tensor.matmul(
                                    qp[:],
                                    wqkv[:, k, 128 * m:128 * (m + 1)],
                                    xT[:, k, QB * n:QB * (n + 1)],
                                    start=(k == 0), stop=(k == 7))
                        return emit

                    def cp():
                        if debug == "prepnq":
                            return
                        qp = state["qp"]
                        if m == 0:
                            nc.vector.tensor_copy(
                                qT[:, QB * n:QB * (n + 1)], qp[:])
                        elif m == 1:
                            nc.vector.tensor_copy(
                                kT[:, QB * n:QB * (n + 1)], qp[:])
                        else:
                            vs = stg.tile([128, QB], bf16, tag="vs",
                                          bufs=2, name=f"vs_{n}")
                            nc.vector.tensor_copy(vs[:], qp[:])
                            cp.vs = vs
                    cp.vs = None
                    return [mm(k0) for k0 in (0, 2, 4, 6)] + [cp]

                def vtr(n, u, qkv_cp):
                    def emit():
                        if debug in ("prepnv", "prepnq"):
                            return
                        t = 4 * n + u
                        vs = qkv_cp.vs
                        # separate PSUM tiles per head: a single tile with
                        # column-split transpose matmuls faults the PE
                        vp0 = ps.tile([128, 64], f32, tag="pA",
                                      name=f"vp0_{t}")
                        vp1 = ps.tile([128, 64], f32, tag="pA",
                                      name=f"vp1_{t}")
                        nc.tensor.matmul(
                            vp0[:], vs[0:64, 128 * u:128 * (u + 1)],
                            iden[0:64, 0:64], start=True, stop=True)
                        nc.tensor.matmul(
                            vp1[:], vs[64:128, 128 * u:128 * (u + 1)],
                            iden[64:128, 64:128], start=True, stop=True)
                        nc.vector.tensor_copy(Vp[:, t, 1:65], vp0[:])
                        nc.vector.tensor_copy(Vp[:, t, 65:129], vp1[:])
                    return emit

                def prep_chunks(n):
                    """Emit-closures for q-block n's qkv prep."""
                    qv = qkv(n, 2)
                    return ([xload(n)] + qkv(n, 0) + qkv(n, 1) + qv
                            + [vtr(n, u, qv[-1]) for u in range(4)])

                class Group:
                    """One (q-block, 2-ktile) attention group, split so the
                    score matmuls can be emitted a group ahead of the
                    exp/mask/AV (software pipeline: PE feeds ACT early)."""

                    def __init__(self, qb, g):
                        self.qb, self.g = qb, g

                    def off(self, kt):
                        d = kt - 4 * self.qb
                        return 128 * d if d >= 0 else 0

                    def scores(self):
                        if debug in ("prep", "prepA", "prepB", "prepnv", "prepnq"):
                            return
                        qb, g = self.qb, self.g
                        self.sps = [ps.tile([128, 2 * QB], f32, tag="sT",
                                            name=f"sp_{qb}_{g}_{h}")
                                    for h in (0, 1)]
                        for h in (0, 1):
                            for u in (0, 1):
                                kt = 2 * g + u
                                o = self.off(kt)
                                nc.tensor.matmul(
                                    self.sps[h][:, QB * u + o:QB * (u + 1)],
                                    kT[64 * h:64 * (h + 1),
                                       128 * kt:128 * (kt + 1)],
                                    qT[64 * h:64 * (h + 1),
                                       QB * qb + o:QB * (qb + 1)],
                                    start=True, stop=True)

                    def post(self, ytps):
                        if debug in ("prep", "prepA", "prepB", "prepnv", "prepnq"):
                            return
                        qb, g = self.qb, self.g
                        # trim the exp only when it saves more than the
                        # extra instruction's access-init cost
                        if debug == "scores":
                            return
                        o0, o1 = self.off(2 * g), self.off(2 * g + 1)
                        for h in (0, 1):
                            pt = stg.tile([128, 2 * QB], bf16, tag="pT",
                                          bufs=4, name=f"pt_{qb}_{g}_{h}")
                            if o0 + o1 > 0:
                                for u in (0, 1):
                                    o = self.off(2 * g + u)
                                    nc.scalar.activation(
                                        pt[:, QB * u + o:QB * (u + 1)],
                                        self.sps[h][:, QB * u + o:
                                                    QB * (u + 1)],
                                        AFT.Exp, scale=0.125)
                            else:
                                nc.scalar.activation(pt[:], self.sps[h][:],
                                                     AFT.Exp, scale=0.125)
                            if debug == "exp":
                                continue
                            for u in (0, 1):
                                kt = 2 * g + u
                                d = kt - 4 * qb
                                o = self.off(kt)
                                if debug != "mask" and d >= 0:
                                    nc.vector.tensor_mul(
                                        pt[:, QB * u + o:QB * (u + 1)],
                                        pt[:, QB * u + o:QB * (u + 1)],
                                        mks[:, d, o:QB])
                                # AV, natural orientation: per q-tile
                                # lhsT = exp-scores [128k, 128q].
                                # ONE psum accumulation group per head-tile
                                # per block: start zeroes the whole 2KB
                                # zero-region (all 4 qt slices), so only the
                                # first matmul starts and only the last stops
                                for r in range(4):
                                    if debug == "mask":
                                        continue
                                    qt_g = 4 * qb + r
                                    if kt > qt_g:
                                        continue
                                    nc.tensor.matmul(
                                        ytps[h][:, r, :],
                                        pt[:, QB * u + 128 * r:
                                           QB * u + 128 * (r + 1)],
                                        Vp[:, kt, 65 * h:65 * (h + 1)],
                                        start=(kt == 0 and r == 0),
                                        stop=(kt == 4 * qb + 3 and r == 3))

                def normalizeA(qb, ytps):
                    if debug in ("prep", "prepA", "prepB", "prepnv", "prepnq", "scores", "exp", "mask", "av"):
                        return None
                    """DVE-only part: reciprocal of the denominators and the
                    normalize multiplies into a per-block SBUF tile.  Runs at
                    the block boundary (must precede the next block's ytp
                    allocation for pool-dep correctness)."""
                    rcp = stg.tile([128, 16], f32, tag="rc", bufs=2,
                                   name=f"rcp_{qb}")
                    # denominators: h0 at col 0, h1 at col 64 of each 65-col
                    # (contiguous per-slice APs: strided cross-slice reads
                    # can leave the PE/DVE psum-bank handoff mistracked)
                    for r in range(4):
                        nc.vector.tensor_copy(rcp[:, r:r + 1],
                                              ytps[0][:, r, 0:1])
                        nc.vector.tensor_copy(rcp[:, 4 + r:5 + r],
                                              ytps[1][:, r, 64:65])
                    nc.vector.reciprocal(rcp[:, 8:16], rcp[:, 0:8])
                    ybk = stg.tile([128, 4, 128], bf16, tag="ybk", bufs=2,
                                   name=f"ybk_{qb}")
                    for r in range(4):
                        nc.vector.tensor_scalar_mul(
                            ybk[:, r, 0:64], ytps[0][:, r, 1:65],
                            rcp[:, 8 + r:9 + r])
                        nc.vector.tensor_scalar_mul(
                            ybk[:, r, 64:128], ytps[1][:, r, 0:64],
                            rcp[:, 12 + r:13 + r])
                    return ybk

                def normalizeB(qb, ybk):
                    """PE transposes + AllToAll staging; deferrable into the
                    next block so the PE stream never stalls on the DVE
                    normalize chain."""
                    if debug in ("attn", "prep", "prepA", "prepB", "prepnv", "prepnq", "scores", "exp", "mask", "av"):
                        return
                    g = qb // 2
                    j0 = 4 * (qb % 2)
                    ytsb = stg.tile([128, 4, 128], bf16, tag="yts", bufs=2,
                                    name=f"ytsb_{qb}")
                    for r in range(4):
                        ytr = ps.tile([128, 128], f32, tag="pA",
                                      name=f"ytr_{qb}_{r}")
                        nc.tensor.matmul(ytr[:], ybk[:, r, :], iden[:],
                                         start=True, stop=True)
                        nc.vector.tensor_copy(ytsb[:, r, :], ytr[:])
                    for r in range(4):
                        nc.sync.dma_start(a2a_in[g][j0 + r], ytsb[:, r, :])

                def collective(g):
                    if debug is not None and debug != "coll":
                        return
                    if use_collective:
                        nc.gpsimd.collective_compute(
                            "AllToAll", mybir.AluOpType.bypass,
                            ins=[a2a_in[g].opt()], outs=[a2a_out[g].opt()],
                            replica_groups=[list(range(N_CORES))])
                    else:
                        nc.sync.dma_start(a2a_out[g][:], a2a_in[g][:])

                def proj(g):
                    """Fine-grained closures: a2a_out load, then the two
                    512-col halves as 2x(4 matmuls)+copy, then the out DMA."""
                    state = {}

                    def load():
                        asb = stg.tile([128, 8, 128], bf16, tag="asb",
                                       bufs=2, name=f"asb_{g}")
                        for c in range(N_CORES):
                            nc.sync.dma_start(asb[:, c, :], a2a_out[g][c])
                        state["asb"] = asb
                        state["ob"] = stg.tile([128, D_MODEL], f32, tag="ob",
                                               bufs=2, name=f"ob_{g}")

                    def mm(n2, c0):
                        def emit():
                            if c0 == 0:
                                state[n2] = ps.tile([128, QB], f32, tag="pA",
                                                    name=f"pp_{g}_{n2}")
                            pp = state[n2]
                            for c in range(c0, c0 + 4):
                                nc.tensor.matmul(
                                    pp[:],
                                    state["asb"][:, c, :],
                                    wpj[:, c, QB * n2:QB * (n2 + 1)],
                                    start=(c == 0), stop=(c == 7))
                        return emit

                    def cp(n2):
                        def emit():
                            nc.vector.tensor_copy(
                                state["ob"][:, QB * n2:QB * (n2 + 1)],
                                state[n2][:])
                        return emit

                    def store(n2):
                        def emit():
                            nc.sync.dma_start(
                                out_d[128 * g:128 * (g + 1),
                                      QB * n2:QB * (n2 + 1)],
                                state["ob"][:, QB * n2:QB * (n2 + 1)])
                        return emit

                    if debug is not None:
                        return []
                    return [load, mm(0, 0), mm(0, 4), cp(0), store(0),
                            mm(1, 0), mm(1, 4), cp(1), store(1)]

                # ---- schedule: flat software-pipelined emission -------
                # Global group list; scores are emitted ONE group ahead of
                # their exp/mask/AV so ACT never waits on braided PE work.
                groups = [Group(qb, g)
                          for qb in range(nQB) for g in range(2 * (qb + 1))]
                first = {qb: next(i for i, gr in enumerate(groups)
                                  if gr.qb == qb) for qb in range(nQB)}

                # startup: finely staggered DMAs so the first exp runs ASAP
                def xpair(j):
                    nc.sync.dma_start(xT[:, 2 * j:2 * j + 2, 0:QB],
                                      x_d[:, 2 * j:2 * j + 2, 0:QB])

                nc.sync.dma_start(wqkv[:, 0:4, :], wq_d[:, 0:4, :])
                xpair(0)
                xpair(1)
                nc.sync.dma_start(wqkv[:, 4:8, :], wq_d[:, 4:8, :])
                xpair(2)
                xpair(3)
                nc.sync.dma_start(iden[:], id_d[:])
                for m in range(4):
                    nc.sync.dma_start(mks[:, m, :], mk_d[m])
                qq, qk, qv = qkv(0, 0), qkv(0, 1), qkv(0, 2)
                for c in qq:
                    c()
                for c in qk:
                    c()
                groups[0].scores()
                for c in qv:
                    c()
                for u in range(4):
                    vtr(0, u, qv[-1])()

                ytps = None
                prev = None                   # previous block's (ytps, qb)
                pend, ci = [], 0
                for i, gr in enumerate(groups):
                    qb = gr.qb
                    nxt = groups[i + 1] if i + 1 < len(groups) else None
                    if nxt is not None and nxt.qb == qb:
                        nxt.scores()
                        nxt = None          # already emitted
                    if i == first[qb]:
                        # block boundary: finish the previous block's
                        # normalize (DVE part now; PE transposes + staging +
                        # collective braided into this block), then allocate
                        # this block's PSUM accumulators
                        npend = []
                        if qb + 1 < nQB:
                            npend += prep_chunks(qb + 1)
                        if prev is not None:
                            pytps, pqb = prev
                            ybk = normalizeA(pqb, pytps)

                            def normB(pqb=pqb, ybk=ybk):
                                normalizeB(pqb, ybk)
                                if pqb % 2 == 1:
                                    collective(pqb // 2)
                            npend.insert(min(7, len(npend)), normB)
                            prev = None
                        while ci < len(pend):   # flush leftovers
                            pend[ci]()
                            ci += 1
                        pend, ci = npend, 0
                        ytps = [ps.tile([128, 4, 65], f32, tag=f"yt{h}",
                                        bufs=1, name=f"yt{h}_{qb}")
                                for h in (0, 1)]
                        if qb == 1:
                            pend += [wpj_load(0), wpj_load(1)]
                        # projection of a2a group g braided into block 2g+4
                        if qb >= 4 and qb % 2 == 0:
                            pend += proj(qb // 2 - 2)
                        nB = 2 * (qb + 1)
                    gr.post(ytps)
                    # at a block boundary, the next block's first scores go
                    # AFTER this post so the AV matmuls aren't head-of-line
                    # blocked behind the scores' sps-slot wait
                    if nxt is not None:
                        nxt.scores()
                    gi = i - first[qb]
                    want = (gi + 1) * len(pend) // nB
                    while ci < want:
                        pend[ci]()
                        ci += 1
                    if i + 1 == len(groups) or groups[i + 1].qb != qb:
                        prev = (ytps, qb)
                # tail: last block's normalize, final AllToAll, projections
                pytps, pqb = prev
                ybk = normalizeA(pqb, pytps)
                normalizeB(pqb, ybk)
                collective(3)
                for c in proj(2):
                    c()
                for c in proj(3):
                    c()
                if debug is not None:
                    dummy = stg.tile([128, D_MODEL], f32, tag="ob", bufs=2,
                                     name="dummy")
                    nc.any.memset(dummy[:], 0.0)
                    for g in range(NG):
                        nc.sync.dma_start(out_d[128 * g:128 * (g + 1), :],
                                          dummy[:])

    if split_waits:
        _split_multi_waits(nc)
    return nc


def make_aux_inputs():
    ident = np.eye(128, dtype=BF16)
    k_idx = np.arange(128)[:, None]
    q_idx = np.arange(QB)[None, :]
    masks = np.stack(
        [((k_idx + 128 * d) <= q_idx).astype(BF16) for d in range(4)], axis=0)
    return ident, masks


def make_in_maps(x, w_qkv, w_proj, seq=SEQ):
    x = np.asarray(x, dtype=np.float32).reshape(seq, D_MODEL)
    w_qkv = np.asarray(w_qkv, dtype=np.float32)
    w_proj = np.asarray(w_proj, dtype=np.float32)
    # [d_model, seq] -> [128, 8, seq] with row = 128*k + p
    xT = np.ascontiguousarray(
        x.T.astype(BF16).reshape(8, 128, seq).transpose(1, 0, 2))
    wp = np.ascontiguousarray(
        w_proj.astype(BF16).reshape(8, 128, D_MODEL).transpose(1, 0, 2))
    ident, masks = make_aux_inputs()
    in_maps = []
    for i in range(N_CORES):
        sl = slice(CPC * i, CPC * (i + 1))
        w_slice = np.concatenate(
            [w_qkv[:, sl], w_qkv[:, D_MODEL:][:, sl],
             w_qkv[:, 2 * D_MODEL:][:, sl]], axis=1)
        ws = np.ascontiguousarray(
            w_slice.astype(BF16).reshape(8, 128, 3 * CPC).transpose(1, 0, 2))
        in_maps.append({
            "xT": xT,
            "w_slice": ws,
            "w_proj": wp,
            "ident": ident,
            "masks": masks,
        })
    return in_maps


_NC_CACHE = {}


def kernel(x, w_qkv, w_proj):
    """Full inputs in, full output out. Shards internally across 8 cores."""
    try:
        import os
        import jax
        jax.config.update("jax_compilation_cache_dir",
                          os.path.expanduser("~/.cache/jax_bass_kernel"))
        jax.config.update("jax_persistent_cache_min_compile_time_secs", 0.0)
    except Exception:
        pass
    from concourse.bass_utils import run_bass_kernel_spmd

    x = np.asarray(x, dtype=np.float32)
    batch = x.shape[0]
    seq = x.shape[1]
    if seq not in _NC_CACHE:
        _NC_CACHE[seq] = build_nc(seq)
    nc = _NC_CACHE[seq]
    in_maps = make_in_maps(x, w_qkv, w_proj, seq=seq)
    res = run_bass_kernel_spmd(nc, in_maps, list(range(N_CORES)))
    # core j's out rows are q-tiles {8g + j : g}, group-major
    out = np.empty((seq // 128, 128, D_MODEL), dtype=np.float32)
    for j in range(N_CORES):
        rj = res.results[j]["out"].reshape(NG, 128, D_MODEL)
        for g in range(NG):
            out[8 * g + j] = rj[g]
    return out.reshape(batch, seq, D_MODEL).astype(np.float32)


# revision 28
# speedup vs baseline: 1.4672x; 1.0112x over previous
"""Causal self-attention (d_model=1024, n_head=16, seq=4096) on 8 trn2 cores.

Sharding: tensor-parallel over heads (2 heads/core) for QKV + attention.
The output shard is q-tile interleaved: core j owns q-tiles {j, 8+j, 16+j,
24+j} (128 rows each), so the head->seq reshard runs as FOUR small
AllToAlls (one per q-tile group), each issued as soon as its two q-blocks
finish -- collectives #0-2 overlap the remaining attention, only #3 is
tail.  Each core projects its 4 q-tiles with the full w_proj; the host
re-interleaves the rows.

Per-core layout (bf16 into the PE, fp32 PSUM accumulation):
  - x^T, w_qkv slice, w_proj are pre-transposed/pre-cast to bf16 on the
    host (pure data movement), so no on-device transposes or dtype
    converts of inputs remain.
  - qkv^T = w_slice.T @ x^T lands in [chan, T] layout: qT/kT are directly
    the lhsT/rhs of the score matmul (scores^T = K Q^T); V' (natural
    orientation, with leading/trailing ones columns for the softmax
    denominators) comes from small PE transposes.
  - softmax without max-subtraction (scores ~ N(0,1): exp cannot overflow
    fp32); AV runs in natural orientation y[q,d] via lhsT=exp-scores
    (M=128 q rows per matmul, N=65), so the denominator falls out as a
    ones-column and normalization is one per-partition-scalar multiply
    after a DVE reciprocal.
  - causal masking: only lower-triangle k-tiles are computed; diagonal
    tiles are masked by a precomputed 0/1 multiply after the exp.
  - emission is braided: prep for block n+1 (x^T DMA/qkv/V') and the
    projection of already-landed AllToAll groups are interleaved between
    the attention groups of q-block n under a shared PSUM pool.
"""

import sys
import types

import numpy as np
import ml_dtypes

D_MODEL = 1024
N_HEAD = 16
SEQ = 4096
N_CORES = 8
D_HEAD = 64
CPC = 128            # channels per core (2 heads x 64)
QB = 512             # attention q-block width
NG = 4               # AllToAll groups (qt-interleaved output shard)
BF16 = ml_dtypes.bfloat16


def _install_compat_patches():
    """Stub antenv.axon_hooks (absent in this container) so
    run_bass_kernel_spmd's trace path degrades instead of ImportError."""
    if "antenv.axon_hooks" not in sys.modules:
        mod = types.ModuleType("antenv.axon_hooks")
        mod.get_axon_ntff_profile_hook = lambda: None
        sys.modules["antenv.axon_hooks"] = mod


def _split_multi_waits(nc):
    """The nix walrus here accepts at most ONE sync-wait per instruction
    (setupSyncWait: 'Too many sync wait commands').  Hoist extra waits onto
    same-engine NoOps inserted immediately before the instruction -- engine
    streams execute in program order, so semantics are unchanged."""
    import concourse.mybir as mybir

    n = 0
    for fn in nc.m.functions:
        for bb in fn.blocks:
            insts = bb.instructions
            out = []
            for inst in insts:
                si = getattr(inst, "sync_info", None)
                waits = list(si.on_wait) if si is not None else []
                if len(waits) > 1:
                    si.on_wait.clear()
                    for w in waits[:-1]:
                        n += 1
                        nop = mybir.InstNoOp(name=f"I-WSPLIT{n}", ins=[], outs=[])
                        nop.engine = inst.engine
                        nop.sync_info = mybir.SyncInfo(on_wait=[w], on_update=[])
                        out.append(nop)
                    si.on_wait.append(waits[-1])
                out.append(inst)
            bb.instructions = out


def build_nc(seq=SEQ, use_collective=True, split_waits=True, debug=None):
    """Build the single-core SPMD program (identical on all 8 cores)."""
    import concourse.bass as bass
    import concourse.mybir as mybir
    from concourse.tile import TileContext

    _install_compat_patches()

    f32 = mybir.dt.float32
    bf16 = mybir.dt.bfloat16
    AFT = mybir.ActivationFunctionType

    nT = seq // 128       # k-tiles
    nQB = seq // QB       # attention q-blocks (8)
    nQT = seq // 128      # q-tiles (32)
    assert nQB == 2 * NG

    nc = bass.Bass("TRN2", target_bir_lowering=False, debug=False,
                   num_devices=N_CORES)
    # host-prepped layouts: [part(128), chunk, free]
    x_d = nc.dram_tensor("xT", [128, 8, seq], bf16, kind="ExternalInput").ap()
    wq_d = nc.dram_tensor("w_slice", [128, 8, 3 * CPC], bf16,
                          kind="ExternalInput").ap()
    wp_d = nc.dram_tensor("w_proj", [128, 8, D_MODEL], bf16,
                          kind="ExternalInput").ap()
    id_d = nc.dram_tensor("ident", [128, 128], bf16, kind="ExternalInput").ap()
    mk_d = nc.dram_tensor("masks", [4, 128, QB], bf16,
                          kind="ExternalInput").ap()
    # out rows: group-major, 128 rows per group = this core's q-tile 8g+j
    out_d = nc.dram_tensor("out", [NG * 128, D_MODEL], f32,
                           kind="ExternalOutput").ap()

    with TileContext(nc) as tc:
        with (
            tc.tile_pool(name="per", bufs=1) as per,
            tc.tile_pool(name="stg", bufs=2) as stg,
            tc.tile_pool(name="dram", bufs=1, space="DRAM") as dram,
        ):
            xT = per.tile([128, 8, seq], bf16)
            qT = per.tile([128, seq], bf16)      # [2 heads x 64 d, T]
            kT = per.tile([128, seq], bf16)
            # V' tiles: [1 | v_h0 | v_h1 | 1]  (denoms ride the AV matmul)
            Vp = per.tile([128, nT, 130], bf16)
            wqkv = per.tile([128, 8, 3 * CPC], bf16)
            wpj = per.tile([128, 8, D_MODEL], bf16)
            iden = per.tile([128, 128], bf16)
            mks = per.tile([128, 4, QB], bf16)

            nc.any.memset(Vp[:, :, 0:1], 1.0)
            nc.any.memset(Vp[:, :, 129:130], 1.0)

            a2a_in = [dram.tile([N_CORES, 128, 128], bf16, name=f"a2a_in{g}")
                      for g in range(NG)]
            a2a_out = [dram.tile([N_CORES, 128, 128], bf16, name=f"a2a_out{g}")
                       for g in range(NG)]

            with (
                tc.tile_pool(name="ps", bufs=2, space="PSUM") as ps,
            ):
                # ---- braided emission ---------------------------------
                # PSUM banks: sT 2x2 + yt0 1 + yt1 1 + pA 2x1 = 8
                def xload(n):
                    def emit():
                        nc.sync.dma_start(xT[:, :, QB * n:QB * (n + 1)],
                                          x_d[:, :, QB * n:QB * (n + 1)])
                    return emit

                def wpj_load(half):
                    def emit():
                        nc.sync.dma_start(
                            wpj[:, 4 * half:4 * (half + 1), :],
                            wp_d[:, 4 * half:4 * (half + 1), :])
                    return emit

                def qkv(n, m):
                    """Split into fine closures: 2 k-chunk matmuls each, then
                    the PSUM->SBUF copy, so braiding never starves ACT."""
                    state = {}

                    def mm(k0):
                        def emit():
                            if debug == "prepnq":
                                return
                            if k0 == 0:
                                state["qp"] = ps.tile([128, QB], f32,
                                                      tag="pA",
                                                      name=f"qp_{n}_{m}")
                            qp = state["qp"]
                            for k in (k0, k0 + 1):
                                nc.tensor.matmul(
                                    qp[:],
                                    wqkv[:, k, 128 * m:128 * (m + 1)],
                                    xT[:, k, QB * n:QB * (n + 1)],
                                    start=(k == 0), stop=(k == 7))
                        return emit

                    def cp():
                        if debug == "prepnq":
                            return
                        qp = state["qp"]
                        if m == 0:
                            nc.vector.tensor_copy(
                                qT[:, QB * n:QB * (n + 1)], qp[:])
                        elif m == 1:
                            nc.vector.tensor_copy(
                                kT[:, QB * n:QB * (n + 1)], qp[:])
                        else:
                            vs = stg.tile([128, QB], bf16, tag="vs",
                                          bufs=2, name=f"vs_{n}")
                            nc.vector.tensor_copy(vs[:], qp[:])
                            cp.vs = vs
                    cp.vs = None
                    return [mm(k0) for k0 in (0, 2, 4, 6)] + [cp]

                def vtr(n, u, qkv_cp):
                    def emit():
                        if debug in ("prepnv", "prepnq"):
                            return
                        t = 4 * n + u
                        vs = qkv_cp.vs
                        # separate PSUM tiles per head: a single tile with
                        # column-split transpose matmuls faults the PE
                        vp0 = ps.tile([128, 64], f32, tag="pA",
                                      name=f"vp0_{t}")
                        vp1 = ps.tile([128, 64], f32, tag="pA",
                                      name=f"vp1_{t}")
                        nc.tensor.matmul(
                            vp0[:], vs[0:64, 128 * u:128 * (u + 1)],
                            iden[0:64, 0:64], start=True, stop=True)
                        nc.tensor.matmul(
                            vp1[:], vs[64:128, 128 * u:128 * (u + 1)],
                            iden[64:128, 64:128], start=True, stop=True)
                        nc.vector.tensor_copy(Vp[:, t, 1:65], vp0[:])
                        nc.vector.tensor_copy(Vp[:, t, 65:129], vp1[:])
                    return emit

                def prep_chunks(n):
                    """Emit-closures for q-block n's qkv prep."""
                    qv = qkv(n, 2)
                    return ([xload(n)] + qkv(n, 0) + qkv(n, 1) + qv
                            + [vtr(n, u, qv[-1]) for u in range(4)])

                class Group:
                    """One (q-block, 2-ktile) attention group, split so the
                    score matmuls can be emitted a group ahead of the
                    exp/mask/AV (software pipeline: PE feeds ACT early)."""

                    def __init__(self, qb, g):
                        self.qb, self.g = qb, g

                    def off(self, kt):
                        d = kt - 4 * self.qb
                        return 128 * d if d >= 0 else 0

                    def scores_h(self, h):
                        if debug in ("prep", "prepA", "prepB", "prepnv", "prepnq"):
                            return
                        qb, g = self.qb, self.g
                        if h == 0:
                            self.sps = [None, None]
                        self.sps[h] = ps.tile([128, 2 * QB], f32, tag="sT",
                                              name=f"sp_{qb}_{g}_{h}")
                        for u in (0, 1):
                            kt = 2 * g + u
                            o = self.off(kt)
                            nc.tensor.matmul(
                                self.sps[h][:, QB * u + o:QB * (u + 1)],
                                kT[64 * h:64 * (h + 1),
                                   128 * kt:128 * (kt + 1)],
                                qT[64 * h:64 * (h + 1),
                                   QB * qb + o:QB * (qb + 1)],
                                start=True, stop=True)

                    def scores(self):
                        self.scores_h(0)
                        self.scores_h(1)

                    def post(self, ytps):
                        self.post_h(0, ytps)
                        self.post_h(1, ytps)

                    def post_h(self, hh, ytps):
                        if debug in ("prep", "prepA", "prepB", "prepnv", "prepnq"):
                            return
                        qb, g = self.qb, self.g
                        # trim the exp only when it saves more than the
                        # extra instruction's access-init cost
                        if debug == "scores":
                            return
                        o0, o1 = self.off(2 * g), self.off(2 * g + 1)
                        for h in (hh,):
                            pt = stg.tile([128, 2 * QB], bf16, tag="pT",
                                          bufs=4, name=f"pt_{qb}_{g}_{h}")
                            if o0 + o1 > 0:
                                for u in (0, 1):
                                    o = self.off(2 * g + u)
                                    nc.scalar.activation(
                                        pt[:, QB * u + o:QB * (u + 1)],
                                        self.sps[h][:, QB * u + o:
                                                    QB * (u + 1)],
                                        AFT.Exp, scale=0.125)
                            else:
                                nc.scalar.activation(pt[:], self.sps[h][:],
                                                     AFT.Exp, scale=0.125)
                            if debug == "exp":
                                continue
                            for u in (0, 1):
                                kt = 2 * g + u
                                d = kt - 4 * qb
                                o = self.off(kt)
                                if debug != "mask" and d >= 0:
                                    nc.vector.tensor_mul(
                                        pt[:, QB * u + o:QB * (u + 1)],
                                        pt[:, QB * u + o:QB * (u + 1)],
                                        mks[:, d, o:QB])
                                # AV, natural orientation: per q-tile
                                # lhsT = exp-scores [128k, 128q].
                                # ONE psum accumulation group per head-tile
                                # per block: start zeroes the whole 2KB
                                # zero-region (all 4 qt slices), so only the
                                # first matmul starts and only the last stops
                                for r in range(4):
                                    if debug == "mask":
                                        continue
                                    qt_g = 4 * qb + r
                                    if kt > qt_g:
                                        continue
                                    nc.tensor.matmul(
                                        ytps[h][:, r, :],
                                        pt[:, QB * u + 128 * r:
                                           QB * u + 128 * (r + 1)],
                                        Vp[:, kt, 65 * h:65 * (h + 1)],
                                        start=(kt == 0 and r == 0),
                                        stop=(kt == 4 * qb + 3 and r == 3))

                def normalizeA(qb, ytps):
                    if debug in ("prep", "prepA", "prepB", "prepnv", "prepnq", "scores", "exp", "mask", "av"):
                        return None
                    """DVE-only part: reciprocal of the denominators and the
                    normalize multiplies into a per-block SBUF tile.  Runs at
                    the block boundary (must precede the next block's ytp
                    allocation for pool-dep correctness)."""
                    rcp = stg.tile([128, 16], f32, tag="rc", bufs=2,
                                   name=f"rcp_{qb}")
                    # denominators: h0 at col 0, h1 at col 64 of each 65-col
                    # (contiguous per-slice APs: strided cross-slice reads
                    # can leave the PE/DVE psum-bank handoff mistracked)
                    for r in range(4):
                        nc.vector.tensor_copy(rcp[:, r:r + 1],
                                              ytps[0][:, r, 0:1])
                        nc.vector.tensor_copy(rcp[:, 4 + r:5 + r],
                                              ytps[1][:, r, 64:65])
                    nc.vector.reciprocal(rcp[:, 8:16], rcp[:, 0:8])
                    ybk = stg.tile([128, 4, 128], bf16, tag="ybk", bufs=2,
                                   name=f"ybk_{qb}")
                    for r in range(4):
                        nc.vector.tensor_scalar_mul(
                            ybk[:, r, 0:64], ytps[0][:, r, 1:65],
                            rcp[:, 8 + r:9 + r])
                        nc.vector.tensor_scalar_mul(
                            ybk[:, r, 64:128], ytps[1][:, r, 0:64],
                            rcp[:, 12 + r:13 + r])
                    return ybk

                def normalizeB(qb, ybk):
                    """PE transposes + AllToAll staging; deferrable into the
                    next block so the PE stream never stalls on the DVE
                    normalize chain."""
                    if debug in ("attn", "prep", "prepA", "prepB", "prepnv", "prepnq", "scores", "exp", "mask", "av"):
                        return
                    g = qb // 2
                    j0 = 4 * (qb % 2)
                    ytsb = stg.tile([128, 4, 128], bf16, tag="yts", bufs=2,
                                    name=f"ytsb_{qb}")
                    for r in range(4):
                        ytr = ps.tile([128, 128], f32, tag="pA",
                                      name=f"ytr_{qb}_{r}")
                        nc.tensor.matmul(ytr[:], ybk[:, r, :], iden[:],
                                         start=True, stop=True)
                        nc.vector.tensor_copy(ytsb[:, r, :], ytr[:])
                    for r in range(4):
                        nc.sync.dma_start(a2a_in[g][j0 + r], ytsb[:, r, :])

                def collective(g):
                    if debug is not None and debug != "coll":
                        return
                    if use_collective:
                        nc.gpsimd.collective_compute(
                            "AllToAll", mybir.AluOpType.bypass,
                            ins=[a2a_in[g].opt()], outs=[a2a_out[g].opt()],
                            replica_groups=[list(range(N_CORES))])
                    else:
                        nc.sync.dma_start(a2a_out[g][:], a2a_in[g][:])

                def proj(g):
                    """Fine-grained closures: a2a_out load, then the two
                    512-col halves as 2x(4 matmuls)+copy, then the out DMA."""
                    state = {}

                    def load():
                        asb = stg.tile([128, 8, 128], bf16, tag="asb",
                                       bufs=2, name=f"asb_{g}")
                        for c in range(N_CORES):
                            nc.sync.dma_start(asb[:, c, :], a2a_out[g][c])
                        state["asb"] = asb
                        state["ob"] = stg.tile([128, D_MODEL], f32, tag="ob",
                                               bufs=2, name=f"ob_{g}")

                    def mm(n2, c0):
                        def emit():
                            if c0 == 0:
                                state[n2] = ps.tile([128, QB], f32, tag="pA",
                                                    name=f"pp_{g}_{n2}")
                            pp = state[n2]
                            for c in range(c0, c0 + 4):
                                nc.tensor.matmul(
                                    pp[:],
                                    state["asb"][:, c, :],
                                    wpj[:, c, QB * n2:QB * (n2 + 1)],
                                    start=(c == 0), stop=(c == 7))
                        return emit

                    def cp(n2):
                        def emit():
                            nc.vector.tensor_copy(
                                state["ob"][:, QB * n2:QB * (n2 + 1)],
                                state[n2][:])
                        return emit

                    def store(n2):
                        def emit():
                            nc.sync.dma_start(
                                out_d[128 * g:128 * (g + 1),
                                      QB * n2:QB * (n2 + 1)],
                                state["ob"][:, QB * n2:QB * (n2 + 1)])
                        return emit

                    if debug is not None:
                        return []
                    return [load, mm(0, 0), mm(0, 4), cp(0), store(0),
                            mm(1, 0), mm(1, 4), cp(1), store(1)]

                # ---- schedule: flat software-pipelined emission -------
                # Global group list; scores are emitted ONE group ahead of
                # their exp/mask/AV so ACT never waits on braided PE work.
                groups = [Group(qb, g)
                          for qb in range(nQB) for g in range(2 * (qb + 1))]
                first = {qb: next(i for i, gr in enumerate(groups)
                                  if gr.qb == qb) for qb in range(nQB)}

                # startup: finely staggered DMAs so the first exp runs ASAP
                def xpair(j):
                    nc.sync.dma_start(xT[:, 2 * j:2 * j + 2, 0:QB],
                                      x_d[:, 2 * j:2 * j + 2, 0:QB])

                nc.sync.dma_start(wqkv[:, 0:2, :], wq_d[:, 0:2, :])
                xpair(0)
                nc.sync.dma_start(wqkv[:, 2:4, :], wq_d[:, 2:4, :])
                xpair(1)
                nc.sync.dma_start(wqkv[:, 4:6, :], wq_d[:, 4:6, :])
                xpair(2)
                nc.sync.dma_start(wqkv[:, 6:8, :], wq_d[:, 6:8, :])
                xpair(3)
                nc.sync.dma_start(iden[:], id_d[:])
                for m in range(4):
                    nc.sync.dma_start(mks[:, m, :], mk_d[m])
                qq, qk, qv = qkv(0, 0), qkv(0, 1), qkv(0, 2)
                for j in range(4):
                    qq[j]()
                    qk[j]()
                qq[4]()
                qk[4]()
                groups[0].scores()
                for c in qv:
                    c()
                for u in range(4):
                    vtr(0, u, qv[-1])()

                ytps = None
                prev = None                   # previous block's (ytps, qb)
                pend, ci = [], 0
                for i, gr in enumerate(groups):
                    qb = gr.qb
                    nxt = groups[i + 1] if i + 1 < len(groups) else None
                    head_pipe = nxt is not None and nxt.qb == qb
                    if i == first[qb]:
                        # block boundary: finish the previous block's
                        # normalize (DVE part now; PE transposes + staging +
                        # collective braided into this block), then allocate
                        # this block's PSUM accumulators
                        npend = []
                        if qb + 1 < nQB:
                            npend += prep_chunks(qb + 1)
                        if prev is not None:
                            pytps, pqb = prev
                            ybk = normalizeA(pqb, pytps)

                            def normB(pqb=pqb, ybk=ybk):
                                normalizeB(pqb, ybk)
                                if pqb % 2 == 1:
                                    collective(pqb // 2)
                            npend.insert(min(7, len(npend)), normB)
                            prev = None
                        while ci < len(pend):   # flush leftovers
                            pend[ci]()
                            ci += 1
                        pend, ci = npend, 0
                        ytps = [ps.tile([128, 4, 65], f32, tag=f"yt{h}",
                                        bufs=1, name=f"yt{h}_{qb}")
                                for h in (0, 1)]
                        if qb == 1:
                            pend += [wpj_load(0), wpj_load(1)]
                        # projection of a2a group g braided into block 2g+4
                        if qb >= 4 and qb % 2 == 0:
                            pend += proj(qb // 2 - 2)
                        nB = 2 * (qb + 1)
                    if i == first[qb]:
                        # shield the first AVs (which wait on the previous
                        # block's normalize chain via the ytp pool) behind a
                        # little braided PE work
                        while ci < min(3, len(pend)):
                            pend[ci]()
                            ci += 1
                    if head_pipe:
                        # steady state: interleave per head so exp of one
                        # head overlaps the other head's scores/AV
                        nxt.scores_h(0)
                        gr.post_h(0, ytps)
                        nxt.scores_h(1)
                        gr.post_h(1, ytps)
                    else:
                        gr.post(ytps)
                        # block boundary: the next block's first scores go
                        # AFTER this post so the AV matmuls aren't
                        # head-of-line blocked on the sps-slot wait
                        if nxt is not None:
                            nxt.scores()
                    gi = i - first[qb]
                    want = (gi + 1) * len(pend) // nB
                    while ci < want:
                        pend[ci]()
                        ci += 1
                    if i + 1 == len(groups) or groups[i + 1].qb != qb:
                        prev = (ytps, qb)
                # tail: last block's normalize, final AllToAll, projections
                pytps, pqb = prev
                ybk = normalizeA(pqb, pytps)
                normalizeB(pqb, ybk)
                collective(3)
                for c in proj(2):
                    c()
                for c in proj(3):
                    c()
                if debug is not None:
                    dummy = stg.tile([128, D_MODEL], f32, tag="ob", bufs=2,
                                     name="dummy")
                    nc.any.memset(dummy[:], 0.0)
                    for g in range(NG):
                        nc.sync.dma_start(out_d[128 * g:128 * (g + 1), :],
                                          dummy[:])

    if split_waits:
        _split_multi_waits(nc)
    return nc


def make_aux_inputs():
    ident = np.eye(128, dtype=BF16)
    k_idx = np.arange(128)[:, None]
    q_idx = np.arange(QB)[None, :]
    masks = np.stack(
        [((k_idx + 128 * d) <= q_idx).astype(BF16) for d in range(4)], axis=0)
    return ident, masks


def make_in_maps(x, w_qkv, w_proj, seq=SEQ):
    x = np.asarray(x, dtype=np.float32).reshape(seq, D_MODEL)
    w_qkv = np.asarray(w_qkv, dtype=np.float32)
    w_proj = np.asarray(w_proj, dtype=np.float32)
    # [d_model, seq] -> [128, 8, seq] with row = 128*k + p
    xT = np.ascontiguousarray(
        x.T.astype(BF16).reshape(8, 128, seq).transpose(1, 0, 2))
    wp = np.ascontiguousarray(
        w_proj.astype(BF16).reshape(8, 128, D_MODEL).transpose(1, 0, 2))
    ident, masks = make_aux_inputs()
    in_maps = []
    for i in range(N_CORES):
        sl = slice(CPC * i, CPC * (i + 1))
        w_slice = np.concatenate(
            [w_qkv[:, sl], w_qkv[:, D_MODEL:][:, sl],
             w_qkv[:, 2 * D_MODEL:][:, sl]], axis=1)
        ws = np.ascontiguousarray(
            w_slice.astype(BF16).reshape(8, 128, 3 * CPC).transpose(1, 0, 2))
        in_maps.append({
            "xT": xT,
            "w_slice": ws,
            "w_proj": wp,
            "ident": ident,
            "masks": masks,
        })
    return in_maps


_NC_CACHE = {}


def kernel(x, w_qkv, w_proj):
    """Full inputs in, full output out. Shards internally across 8 cores."""
    try:
        import os
        import jax
        jax.config.update("jax_compilation_cache_dir",
                          os.path.expanduser("~/.cache/jax_bass_kernel"))
        jax.config.update("jax_persistent_cache_min_compile_time_secs", 0.0)
    except Exception:
        pass
    from concourse.bass_utils import run_bass_kernel_spmd

    x = np.asarray(x, dtype=np.float32)
    batch = x.shape[0]
    seq = x.shape[1]
    if seq not in _NC_CACHE:
        _NC_CACHE[seq] = build_nc(seq)
    nc = _NC_CACHE[seq]
    in_maps = make_in_maps(x, w_qkv, w_proj, seq=seq)
    res = run_bass_kernel_spmd(nc, in_maps, list(range(N_CORES)))
    # core j's out rows are q-tiles {8g + j : g}, group-major
    out = np.empty((seq // 128, 128, D_MODEL), dtype=np.float32)
    for j in range(N_CORES):
        rj = res.results[j]["out"].reshape(NG, 128, D_MODEL)
        for g in range(NG):
            out[8 * g + j] = rj[g]
    return out.reshape(batch, seq, D_MODEL).astype(np.float32)
